# revision 51
# baseline (speedup 1.0000x reference)
"""Trainium2 Bass kernel for multi-head attention (B=4, NQ=NK=2048, E=1024, H=16).

Sharding: 8 cores = 4 batches x 2 head-groups (8 heads each). Each core
computes its head-group's attention and a partial output projection; the
host sums the two partials per batch and adds bo.

Structure (vs the original baseline):
  - Inputs ship host-pre-transposed (qT/kT/vT bf16), so all SBUF loads are
    plain batched DMAs - no xbar transposes and ~100 fewer DMA issues.
  - exp runs entirely on the ACT engine as exp(x*scale - 2) (the -2 cancels
    in the softmax normalization): 256 instructions of [128,1024], reading
    score psum tiles directly.
  - All psum evacuations (q/k bias-add, v copy, P@V copy, out copy) run on
    the DVE so the ACT engine stays dedicated to exp.
  - The jq loop is software-pipelined over 16 (jq, head-pair) slots: each
    slot emits its 16 QK matmuls, the previous slot's P@V + evac + 1/l
    DRAM-bounce launch, the 1/l multiply of the slot before that (so the
    bounce DMA latency never blocks the DVE), the next jq's q-projection
    chunk, and an output-projection chunk of the previous jq.
  - vh carries a ones-column so the P@V matmul's row 64 accumulates the
    softmax denominator for free.
"""

import math
from contextlib import ExitStack

import numpy as np

import concourse.bass as bass
import concourse.mybir as mybir
import concourse.tile as tile
from concourse.vector_clock import ScopedClock

F32R = mybir.dt.float32r
F32 = mybir.dt.float32
BF16 = mybir.dt.bfloat16

B = 4
NQ = 2048
NK = 2048
E = 1024
H = 16
HD = 64
INT = 1024
IL = INT // 2          # per-core internal dim = 512
HL = H // 2            # heads per core = 8
N_CORES = 8
EXP_SHIFT = 2.0        # exp(x - EXP_SHIFT); cancels in normalization


class _SplitDrainTC(tile.TileContext):
    """TileContext whose tail drain chains single-wait Drain instructions.

    The walrus build here rejects Drain instructions carrying more than one
    semaphore wait ("Too many sync wait commands"), while the stock Tile
    tail-drain waits on the whole vector clock in one instruction.
    """

    def _drain_and_barrier(self, tick_clock, wait_clock):
        drain_inst = self.nc.sync.drain()
        wait_clock.add_sem_waits(
            drain_inst.ins, ScopedClock({None: tick_clock.global_clock})
        )
        si = drain_inst.ins.sync_info
        waits = list(si.on_wait) if si and si.on_wait else []
        if len(waits) > 1:
            drain_inst.ins.sync_info = mybir.SyncInfo(
                on_wait=waits[:1], on_update=list(si.on_update or [])
            )
            for i in range(1, len(waits)):
                extra = self.nc.sync.drain()
                extra.ins.sync_info = mybir.SyncInfo(
                    on_wait=waits[i : i + 1], on_update=[]
                )
        self.nc.all_engine_barrier()
        assert self.sems is not None
        popped = self.nc._tile_sem_poison_stack.pop()
        assert popped is self._sem_poison
        self.nc.clear_and_free_semaphores(list(self.sems.allocated().values()))
        self.nc.all_engine_barrier()


def _split_waits(nc, maxw=1):
    """Hoist excess semaphore waits onto same-engine NoOps.

    This walrus build rejects instructions carrying more than one sem wait
    ("Too many sync wait commands"), while Tile attaches the full required
    wait set to each instruction. Same-engine program order makes the
    preceding NoOp waits equivalent.
    """
    for fn in nc.m.functions:
        for blk in fn.blocks:
            insts = list(blk.instructions)
            out = []
            changed = False
            for inst in insts:
                si = inst.sync_info
                waits = list(si.on_wait) if si and si.on_wait else []
                if len(waits) > maxw:
                    changed = True
                    extra, keep = waits[:-maxw], waits[-maxw:]
                    for w in extra:
                        out.append(
                            mybir.InstNoOp(
                                name=nc.get_next_instruction_name(),
                                ins=[],
                                outs=[],
                                engine=inst.engine,
                                sync_info=mybir.SyncInfo(
                                    on_wait=[w], on_update=[]
                                ),
                                bass_nofuse=True,
                            )
                        )
                    inst.sync_info = mybir.SyncInfo(
                        on_wait=keep, on_update=list(si.on_update or [])
                    )
                out.append(inst)
            if changed:
                blk.instructions = out


def _flat(ap3, n):
    """Collapse a contiguous multi-free-dim AP slice to [parts, n]."""
    return bass.AP(tensor=ap3.tensor, offset=ap3.offset,
                   ap=[list(ap3.ap[0]), [1, n]])


def build_nc(
    nq=NQ,
    nk=NK,
    split_waits=True,
    xt_bufs=3,
    exp_bufs=10,
    sp_bufs=2,
    av_bufs=2,
    op_bufs=2,
    qsl_bufs=2,
    qtb_bufs=3,
    atn_bufs=8,
    rl_bufs=3,
    osb_bufs=2,
    warm_mms=13,
):
    """Build the per-core Bass module (SPMD; all cores run this program)."""
    nc = bass.Bass()

    q_h = nc.declare_dram_parameter("qT", [E, nq], BF16, isOutput=False)
    k_h = nc.declare_dram_parameter("kT", [E, nk], BF16, isOutput=False)
    v_h = nc.declare_dram_parameter("vT", [E, nk], BF16, isOutput=False)
    wq_h = nc.declare_dram_parameter("wq", [E, IL], BF16, isOutput=False)
    wk_h = nc.declare_dram_parameter("wk", [E, IL], BF16, isOutput=False)
    wv_h = nc.declare_dram_parameter("wv", [E, IL], BF16, isOutput=False)
    bq_h = nc.declare_dram_parameter("bq", [IL], F32, isOutput=False)
    bk_h = nc.declare_dram_parameter("bk", [IL], F32, isOutput=False)
    bv_h = nc.declare_dram_parameter("bv", [IL], F32, isOutput=False)
    wo_h = nc.declare_dram_parameter("wo", [IL, E], BF16, isOutput=False)
    out_h = nc.declare_dram_parameter("out", [nq, E], BF16, isOutput=True)

    EC = E // 128        # 8 E-chunks
    IB = IL // 128       # 4 INT-chunks (head pairs)
    KB = nk // 128       # 16 key blocks
    SC = nk // 512       # 4 key chunks (of 4 kb each)
    KPC = 4              # kb per chunk
    JQ = nq // 512
    SCALE = 1.0 / math.sqrt(HD)
    Exp = mybir.ActivationFunctionType.Exp

    with _SplitDrainTC(nc) as tc, ExitStack() as top:
        singles = top.enter_context(tc.tile_pool(name="singles", bufs=1))
        persist = top.enter_context(tc.tile_pool(name="persist", bufs=1))
        qtb_p = top.enter_context(tc.tile_pool(name="qtb", bufs=qtb_bufs))
        qsl_p = top.enter_context(tc.tile_pool(name="qsl", bufs=qsl_bufs))
        rld_p = top.enter_context(tc.tile_pool(name="rld", bufs=8, space="DRAM"))
        spsum = top.enter_context(
            tc.tile_pool(name="spsum", bufs=sp_bufs, space="PSUM"))
        avpsum = top.enter_context(
            tc.tile_pool(name="avpsum", bufs=av_bufs, space="PSUM"))
        opsum = top.enter_context(
            tc.tile_pool(name="opsum", bufs=op_bufs, space="PSUM"))

        # biases as [128 part, chunk] per-partition columns
        bq_sb = singles.tile([128, IB], F32)
        bk_sb = singles.tile([128, IB], F32)
        bv_sb = singles.tile([128, IB], F32)

        def load_biases():
            for b_sb, b_h in ((bq_sb, bq_h), (bk_sb, bk_h), (bv_sb, bv_h)):
                nc.scalar.dma_start(
                    out=b_sb, in_=b_h.ap().rearrange("(c p) -> p c", p=128)
                )
        negs = singles.tile([128, 1], F32)
        nc.vector.memset(negs, -EXP_SHIFT)
        ones64 = singles.tile([1, 64], BF16)
        nc.vector.memset(ones64, 1.0)
        warm = singles.tile([128, 512], BF16)
        nc.vector.memset(warm[:, 0:1], 0.0)
        wps = spsum.tile([128, 1024], F32, tag="s")
        for i in range(warm_mms):
            nc.tensor.matmul(wps[:, 0:512], warm[:, 0:128], warm,
                             start=True, stop=True)

        wqb = persist.tile([128, EC, IL], BF16)

        def load_wqb():
            nc.sync.dma_start(
                out=wqb, in_=wq_h.ap().rearrange("(c p) i -> p c i", p=128)
            )
        wo_sb = persist.tile([128, IB, E], BF16)

        def load_wo_chunk(ib):
            nc.sync.dma_start(
                out=wo_sb[:, ib, :],
                in_=wo_h.ap().rearrange("(c p) e -> p c e", p=128)[:, ib, :],
            )
        # k^T: [(h2, hd) part, head-pair, k] per sc chunk
        khTs = [persist.tile([128, IB, 512], BF16, name=f"khT_{i}")
                for i in range(SC)]
        # v: [k part, head, kb-in-chunk, 65] (col 64 = ones -> denominator)
        vhs = [persist.tile([128, HL, KPC, 65], BF16, name=f"vh_{i}")
               for i in range(SC)]
        for t in vhs:
            nc.vector.memset(t[:, :, :, 64:65], 1.0)

        qtb_tiles = {}

        def load_qTb(jq):
            qtb_tiles[jq] = qtb_p.tile([128, EC, 512], BF16, tag="qtb",
                                       name=f"qTb{jq}")
            eng = nc.sync
            eng.dma_start(
                out=qtb_tiles[jq],
                in_=q_h.ap().rearrange("(c p) n -> p c n", p=128)[
                    :, :, jq * 512 : jq * 512 + 512],
            )

        qsl_tiles = {}

        def qproj_chunk(jq, ib):
            """q-projection chunk ib of 512-q block jq -> qsl bf16 (+bias)."""
            if jq not in qsl_tiles:
                qsl_tiles[jq] = qsl_p.tile([128, IB, 512], BF16, tag="qsl",
                                           name=f"qsl{jq}")
            qsl = qsl_tiles[jq]
            ps = opsum.tile([128, 512], F32, tag="o")
            for ec in range(EC):
                nc.tensor.matmul(
                    ps,
                    wqb[:, ec, ib * 128 : ib * 128 + 128],
                    qtb_tiles[jq][:, ec, :],
                    start=(ec == 0),
                    stop=(ec == EC - 1),
                )
            nc.vector.tensor_scalar_add(
                out=qsl[:, ib, :], in0=ps, scalar1=bq_sb[:, ib : ib + 1])

        # ---------------- phase B: k/v projections ----------------
        with ExitStack() as phb:
            wpool = phb.enter_context(tc.tile_pool(name="weights", bufs=1))
            # wk in two ec-halves so the first kproj can start after half
            # the weight + half the first k chunk have landed
            wk_h1 = wpool.tile([128, EC // 2, IL], BF16)
            wk_h2 = wpool.tile([128, EC // 2, IL], BF16)
            wv_sb = wpool.tile([128, EC, IL], BF16)
            xt_p = phb.enter_context(tc.tile_pool(name="xT", bufs=xt_bufs))
            xth_p = phb.enter_context(tc.tile_pool(name="xTh", bufs=2))

            def load_w(w_sb, w_h):
                nc.sync.dma_start(
                    out=w_sb, in_=w_h.ap().rearrange("(c p) i -> p c i", p=128)
                )

            def load_w_half(w_sb, w_h, half):
                nc.sync.dma_start(
                    out=w_sb,
                    in_=w_h.ap().rearrange("(c p) i -> p c i", p=128)[
                        :, half * (EC // 2) : (half + 1) * (EC // 2), :],
                )

            def load_xts(src_h, sc):
                xt = xt_p.tile([128, EC, 512], BF16, tag="xt")
                nc.sync.dma_start(
                    out=xt,
                    in_=src_h.ap().rearrange("(c p) n -> p c n", p=128)[
                        :, :, sc * 512 : sc * 512 + 512],
                )
                return [xt[:, ec, :] for ec in range(EC)]

            def load_xts_halves(src_h, sc):
                parts = []
                for half in range(2):
                    xt = xth_p.tile([128, EC // 2, 512], BF16, tag="xth")
                    nc.sync.dma_start(
                        out=xt,
                        in_=src_h.ap().rearrange("(c p) n -> p c n", p=128)[
                            :, half * (EC // 2) : (half + 1) * (EC // 2),
                            sc * 512 : sc * 512 + 512],
                    )
                    parts.extend(xt[:, i, :] for i in range(EC // 2))
                return parts

            def kproj_chunk(sc, ib, xts):
                ps = opsum.tile([128, 512], F32, tag="o")
                for ec in range(EC):
                    wk_t = wk_h1 if ec < EC // 2 else wk_h2
                    nc.tensor.matmul(
                        ps,
                        wk_t[:, ec % (EC // 2), ib * 128 : ib * 128 + 128],
                        xts[ec],
                        start=(ec == 0),
                        stop=(ec == EC - 1),
                    )
                nc.vector.tensor_scalar_add(
                    out=khTs[sc][:, ib, :], in0=ps,
                    scalar1=bk_sb[:, ib : ib + 1])

            def vproj_chunk(sc, sb, xts):
                ps = opsum.tile([128, 512], F32, tag="o")
                for ec in range(EC):
                    nc.tensor.matmul(
                        ps,
                        xts[ec][:, sb * 128 : sb * 128 + 128],
                        wv_sb[:, ec, :],
                        start=(ec == 0),
                        stop=(ec == EC - 1),
                    )
                nc.vector.tensor_copy(
                    out=vhs[sc][:, :, sb, 0:64],
                    in_=ps.rearrange("p (h d) -> p h d", h=HL),
                )

            # DMA order: wk/kxt(0) in halves -> wv -> vxt(0) -> wqb/qTb(0)
            load_w_half(wk_h1, wk_h, 0)
            load_biases()
            for sc in range(SC):
                if sc == 0:
                    kxts = load_xts_halves(k_h, 0)
                    load_w_half(wk_h2, wk_h, 1)
                    load_w(wv_sb, wv_h)
                else:
                    kxts = load_xts(k_h, sc)
                vxts = load_xts(v_h, sc)
                if sc == 0:
                    load_wqb()
                    load_qTb(0)
                for ib in range(IB):
                    kproj_chunk(sc, ib, kxts)
                for sb in range(KPC):
                    vproj_chunk(sc, sb, vxts)
                if sc == 0:
                    for ib in range(IB):
                        qproj_chunk(0, ib)

        # ---------------- attention + output projection ----------------
        exp_p = top.enter_context(tc.tile_pool(name="exps", bufs=exp_bufs))
        atn_p = top.enter_context(tc.tile_pool(name="atn", bufs=atn_bufs))
        rl_p = top.enter_context(tc.tile_pool(name="rl", bufs=rl_bufs))
        out_p = top.enter_context(tc.tile_pool(name="outsb", bufs=osb_bufs))

        exp_tiles = {}
        atn_tiles = {}
        av_state = {}

        def qk_emit(jq, p_, kb):
            """Scores^T for both heads of pair p_ at key block kb, then exp."""
            sc, kk = kb // KPC, kb % KPC
            key = (jq, p_, sc)
            if key not in exp_tiles:
                exp_tiles[key] = exp_p.tile([128, KPC, 2, 512], BF16, tag="e",
                                            name=f"e{jq}_{p_}_{sc}")
            et = exp_tiles[key]
            ps = spsum.tile([128, 1024], F32, tag="s")
            qsl = qsl_tiles[jq]
            for h2 in (0, 1):
                hp = slice(h2 * 64, h2 * 64 + 64)
                nc.tensor.matmul(
                    ps[:, h2 * 512 : h2 * 512 + 512],
                    khTs[sc][hp, p_, kk * 128 : kk * 128 + 128],
                    qsl[hp, p_, :],
                    start=True,
                    stop=True,
                )
            nc.scalar.activation(out=_flat(et[:, kk, :, :], 1024), in_=ps,
                                 func=Exp, scale=SCALE, bias=negs[:, 0:1])

        av_psums = {}

        def av_stream(jq, p_, sc):
            """P@V matmuls for one key chunk, accumulating into held psums
            (used for the final pair so its AV rides inside the last slot)."""
            if (jq, p_) not in av_psums:
                av_psums[(jq, p_)] = [
                    avpsum.tile([65, 512], F32, tag="av",
                                name=f"avs{jq}_{p_}_{h2}")
                    for h2 in (0, 1)]
            for h2 in (0, 1):
                h = 2 * p_ + h2
                et = exp_tiles[(jq, p_, sc)]
                for kk in range(KPC):
                    nc.tensor.matmul(
                        av_psums[(jq, p_)][h2],
                        vhs[sc][:, h, kk, 0:65],
                        et[:, kk, h2, :],
                        start=(sc == 0 and kk == 0),
                        stop=(sc == SC - 1 and kk == KPC - 1),
                    )

        def av_start(jq, p_, bcast_pe=False):
            """P@V (ones-augmented) for both heads; evac + launch the 1/l
            DRAM-bounce broadcast. The multiply happens in av_finish two
            slots later so the DMA latency never stalls the DVE."""
            avsbs = []
            streamed = av_psums.pop((jq, p_), None)
            rl1 = rl_p.tile([1, 1024], BF16, tag="rl1")
            for h2 in (0, 1):
                h = 2 * p_ + h2
                if streamed is not None:
                    ps = streamed[h2]
                else:
                    ps = avpsum.tile([65, 512], F32, tag="av")
                    for kb in range(KB):
                        sc, kk = kb // KPC, kb % KPC
                        et = exp_tiles[(jq, p_, sc)]
                        nc.tensor.matmul(
                            ps,
                            vhs[sc][:, h, kk, 0:65],
                            et[:, kk, h2, :],
                            start=(kb == 0),
                            stop=(kb == KB - 1),
                        )
                # 1/l of the denom row first (both heads share one bounce
                # tile) so the broadcast launches before the bulk evac
                with nc.allow_low_precision(reason="1/l broadcast in bf16"):
                    nc.vector.reciprocal(
                        out=rl1[:, h2 * 512 : h2 * 512 + 512],
                        in_=ps[64:65, :])
                avsb = rl_p.tile([65, 512], BF16, tag="avsb")
                nc.vector.tensor_copy(out=avsb, in_=ps)
                avsbs.append(avsb)
            if bcast_pe:
                # tail path: broadcast 1/l over d with 1-row PE matmuls so
                # the final normalize never waits on a DRAM round-trip; the
                # av_finish multiply reads the factors straight from psum.
                # Output banks come from the (idle) opsum pool so nothing
                # waits on the exp stream to free a score buffer.
                rt = [opsum.tile([128, 512], F32, tag="o", name=f"rb{hf}")
                      for hf in range(2)]
                for hf in range(2):
                    nc.tensor.matmul(rt[hf][0:64, :],
                                     ones64, rl1[:, hf * 512 : hf * 512 + 512],
                                     start=True, stop=True)
                rlb = rt
            else:
                # bounce through DRAM; stride-0 partition AP broadcasts over d
                rld = rld_p.tile([1, 1024], BF16, tag="rld")
                nc.sync.dma_start(out=rld, in_=rl1)
                rlb = rl_p.tile([64, 1024], BF16, tag="rlb")
                nc.sync.dma_start(
                    out=rlb,
                    in_=bass.AP(
                        tensor=rld.tensor,
                        offset=rld.offset,
                        ap=[[0, 64]] + list(rld.ap)[1:],
                    ),
                )
            av_state[(jq, p_)] = (avsbs, rlb)
            for sc in range(SC):
                exp_tiles.pop((jq, p_, sc), None)

        def av_finish(jq, p_):
            atn = atn_p.tile([128, 512], BF16, tag="a", name=f"atn{jq}_{p_}")
            atn_tiles[(jq, p_)] = atn
            avsbs, rlb = av_state.pop((jq, p_))
            for h2, avsb in enumerate(avsbs):
                hrows = slice(h2 * 64, h2 * 64 + 64)
                fac = (rlb[h2][0:64, :] if isinstance(rlb, list)
                       else rlb[:, h2 * 512 : h2 * 512 + 512])
                nc.vector.tensor_mul(atn[hrows, :], avsb[0:64, :], fac)
            # + bv (deferred v-projection bias: P @ (vh + bv) = P@vh + l*bv,
            #   so atn just gains bv after normalize)
            nc.vector.tensor_scalar_add(
                out=atn, in0=atn, scalar1=bv_sb[:, p_ : p_ + 1])

        def oproj_chunk(jq, m2, split_store=False):
            m = jq * 4 + m2
            osb = out_p.tile([128, 1024], BF16, tag="osb")
            for half in range(2):
                ps = opsum.tile([128, 512], F32, tag="o")
                for ic in range(IB):
                    nc.tensor.matmul(
                        ps,
                        atn_tiles[(jq, ic)][:, m2 * 128 : m2 * 128 + 128],
                        wo_sb[:, ic, half * 512 : half * 512 + 512],
                        start=(ic == 0),
                        stop=(ic == IB - 1),
                    )
                nc.vector.tensor_copy(
                    out=osb[:, half * 512 : half * 512 + 512], in_=ps)
                if split_store:
                    nc.sync.dma_start(
                        out=out_h.ap()[m * 128 : m * 128 + 128,
                                       half * 512 : half * 512 + 512],
                        in_=osb[:, half * 512 : half * 512 + 512])
            if not split_store:
                nc.sync.dma_start(
                    out=out_h.ap()[m * 128 : m * 128 + 128, :], in_=osb)

        # pipelined jq loop over 16 linear slots t=(jq,p_): each slot
        # interleaves its 16 QK+exp with deferred work scheduled by slot
        # index: av_start one slot after its QK, av_finish two slots after
        # (hiding the 1/l DMA bounce), qproj for the next jq, and oproj
        # chunks after the owning jq's last av_finish.
        NSLOT = 4 * JQ
        fills = {}

        def at(t, fn):
            fills.setdefault(t, []).append(fn)

        load_qTb(1)
        for ib in range(IB):
            at(1 + ib, lambda ib=ib: load_wo_chunk(ib))
        at(2, lambda: load_qTb(2))
        at(6, lambda: load_qTb(3))

        for jq in range(JQ):
            for p_ in range(4):
                t = 4 * jq + p_
                fstart = (lambda jq=jq, p=p_, b=(t == NSLOT - 1):
                          av_start(jq, p, bcast_pe=b))
                fstart._av_last = 1
                at(t + 1, fstart)
                fin = t + 1 if t + 2 > NSLOT else t + 2
                at(fin, lambda jq=jq, p=p_: av_finish(jq, p))
            if jq + 1 < JQ:
                for ib in range(IB):
                    at(4 * jq + ib, lambda jq=jq + 1, ib=ib: qproj_chunk(jq, ib))
            for m2 in range(4):
                # last av_finish of jq lands at 4*jq+5; oproj from there
                # (m2=3 shares slot 4jq+7 so less lands in the tail)
                at(4 * jq + 5 + m2,
                   lambda jq=jq, m2=m2, ss=(jq == JQ - 1 and m2 == 3):
                   oproj_chunk(jq, m2, split_store=ss))

        qk_sched = {t: [(t // 4, t % 4, kb) for kb in range(KB)]
                    for t in range(NSLOT)}

        for t in range(NSLOT):
            jq, p_ = t // 4, t % 4
            todo = fills.pop(t, [])
            # av_start last within the slot: its P@V then has the whole
            # slot's exp production behind it before the next slot's QK
            todo.sort(key=lambda f: getattr(f, '_av_last', 0))
            items = qk_sched[t]
            if t == NSLOT - 1:
                # final slot: QK first so its exps drain before the tail
                for item in items:
                    qk_emit(*item)
                for fn in todo:
                    fn()
                continue
            fi = 0
            for ki, item in enumerate(items):
                qk_emit(*item)
                while (fi < len(todo)
                       and ki + 1 >= (fi + 1) * len(items) // (len(todo) + 1)):
                    todo[fi]()
                    fi += 1
            while fi < len(todo):
                todo[fi]()
                fi += 1
        for t in sorted(fills):
            for fn in fills.pop(t):
                fn()

    if split_waits:
        _split_waits(nc)
    return nc


_CACHED = {}


def _get_nc(nq=NQ, nk=NK):
    key = (nq, nk)
    if key not in _CACHED:
        _CACHED[key] = build_nc(nq, nk)
    return _CACHED[key]


def shard_inputs(q, k, v, wq, bq, wk, bk, wv, bv, wo):
    """8 per-core input maps: core c -> (batch c//2, head-group c%2)."""
    import ml_dtypes

    bf = ml_dtypes.bfloat16
    in_maps = []
    for c in range(N_CORES):
        b, g = c // 2, c % 2
        sl = slice(g * IL, (g + 1) * IL)
        in_maps.append(
            {
                "qT": np.ascontiguousarray(q[b].T).astype(bf),
                "kT": np.ascontiguousarray(k[b].T).astype(bf),
                "vT": np.ascontiguousarray(v[b].T).astype(bf),
                "wq": np.ascontiguousarray(wq[:, sl]).astype(bf),
                "wk": np.ascontiguousarray(wk[:, sl]).astype(bf),
                "wv": np.ascontiguousarray(wv[:, sl]).astype(bf),
                "bq": np.ascontiguousarray(bq[sl]),
                "bk": np.ascontiguousarray(bk[sl]),
                "bv": np.ascontiguousarray(bv[sl]),
                "wo": np.ascontiguousarray(wo[sl, :]).astype(bf),
            }
        )
    return in_maps


def kernel(q, k, v, wq, bq, wk, bk, wv, bv, wo, bo, _trace=False):
    from concourse.bass_utils import run_bass_kernel_spmd

    q, k, v = (np.asarray(x, np.float32) for x in (q, k, v))
    wq, bq, wk, bk, wv, bv, wo, bo = (
        np.asarray(x, np.float32) for x in (wq, bq, wk, bk, wv, bv, wo, bo)
    )
    nc = _get_nc()
    in_maps = shard_inputs(q, k, v, wq, bq, wk, bk, wv, bv, wo)
    try:
        res = run_bass_kernel_spmd(
            nc, in_maps, core_ids=list(range(N_CORES)), trace=_trace
        )
    except Exception:
        if not _trace:
            raise
        import traceback

        traceback.print_exc()
        print("trace run failed; retrying without trace", flush=True)
        res = run_bass_kernel_spmd(
            nc, in_maps, core_ids=list(range(N_CORES)), trace=False
        )
    parts = np.stack(
        [np.asarray(res.results[c]["out"], np.float32) for c in range(N_CORES)]
    )
    out = parts.reshape(B, 2, NQ, E).sum(axis=1) + bo[None, None, :]
    if _trace:
        kernel.last_results = res
    return out.astype(np.float32)


# revision 52
# speedup vs baseline: 1.0032x; 1.0032x over previous
"""Trainium2 Bass kernel for multi-head attention (B=4, NQ=NK=2048, E=1024, H=16).

Sharding: 8 cores = 4 batches x 2 head-groups (8 heads each). Each core
computes its head-group's attention and a partial output projection; the
host sums the two partials per batch and adds bo.

Structure (vs the original baseline):
  - Inputs ship host-pre-transposed (qT/kT/vT bf16), so all SBUF loads are
    plain batched DMAs - no xbar transposes and ~100 fewer DMA issues.
  - exp runs entirely on the ACT engine as exp(x*scale - 2) (the -2 cancels
    in the softmax normalization): 256 instructions of [128,1024], reading
    score psum tiles directly.
  - All psum evacuations (q/k bias-add, v copy, P@V copy, out copy) run on
    the DVE so the ACT engine stays dedicated to exp.
  - The jq loop is software-pipelined over 16 (jq, head-pair) slots: each
    slot emits its 16 QK matmuls, the previous slot's P@V + evac + 1/l
    DRAM-bounce launch, the 1/l multiply of the slot before that (so the
    bounce DMA latency never blocks the DVE), the next jq's q-projection
    chunk, and an output-projection chunk of the previous jq.
  - vh carries a ones-column so the P@V matmul's row 64 accumulates the
    softmax denominator for free.
"""

import math
from contextlib import ExitStack

import numpy as np

import concourse.bass as bass
import concourse.mybir as mybir
import concourse.tile as tile
from concourse.vector_clock import ScopedClock

F32R = mybir.dt.float32r
F32 = mybir.dt.float32
BF16 = mybir.dt.bfloat16

B = 4
NQ = 2048
NK = 2048
E = 1024
H = 16
HD = 64
INT = 1024
IL = INT // 2          # per-core internal dim = 512
HL = H // 2            # heads per core = 8
N_CORES = 8
EXP_SHIFT = 2.0        # exp(x - EXP_SHIFT); cancels in normalization


class _SplitDrainTC(tile.TileContext):
    """TileContext whose tail drain chains single-wait Drain instructions.

    The walrus build here rejects Drain instructions carrying more than one
    semaphore wait ("Too many sync wait commands"), while the stock Tile
    tail-drain waits on the whole vector clock in one instruction.
    """

    def _drain_and_barrier(self, tick_clock, wait_clock):
        drain_inst = self.nc.sync.drain()
        wait_clock.add_sem_waits(
            drain_inst.ins, ScopedClock({None: tick_clock.global_clock})
        )
        si = drain_inst.ins.sync_info
        waits = list(si.on_wait) if si and si.on_wait else []
        if len(waits) > 1:
            drain_inst.ins.sync_info = mybir.SyncInfo(
                on_wait=waits[:1], on_update=list(si.on_update or [])
            )
            for i in range(1, len(waits)):
                extra = self.nc.sync.drain()
                extra.ins.sync_info = mybir.SyncInfo(
                    on_wait=waits[i : i + 1], on_update=[]
                )
        self.nc.all_engine_barrier()
        assert self.sems is not None
        popped = self.nc._tile_sem_poison_stack.pop()
        assert popped is self._sem_poison
        self.nc.clear_and_free_semaphores(list(self.sems.allocated().values()))
        self.nc.all_engine_barrier()


def _split_waits(nc, maxw=1):
    """Hoist excess semaphore waits onto same-engine NoOps.

    This walrus build rejects instructions carrying more than one sem wait
    ("Too many sync wait commands"), while Tile attaches the full required
    wait set to each instruction. Same-engine program order makes the
    preceding NoOp waits equivalent.
    """
    for fn in nc.m.functions:
        for blk in fn.blocks:
            insts = list(blk.instructions)
            out = []
            changed = False
            for inst in insts:
                si = inst.sync_info
                waits = list(si.on_wait) if si and si.on_wait else []
                if len(waits) > maxw:
                    changed = True
                    extra, keep = waits[:-maxw], waits[-maxw:]
                    for w in extra:
                        out.append(
                            mybir.InstNoOp(
                                name=nc.get_next_instruction_name(),
                                ins=[],
                                outs=[],
                                engine=inst.engine,
                                sync_info=mybir.SyncInfo(
                                    on_wait=[w], on_update=[]
                                ),
                                bass_nofuse=True,
                            )
                        )
                    inst.sync_info = mybir.SyncInfo(
                        on_wait=keep, on_update=list(si.on_update or [])
                    )
                out.append(inst)
            if changed:
                blk.instructions = out


def _flat(ap3, n):
    """Collapse a contiguous multi-free-dim AP slice to [parts, n]."""
    return bass.AP(tensor=ap3.tensor, offset=ap3.offset,
                   ap=[list(ap3.ap[0]), [1, n]])


def build_nc(
    nq=NQ,
    nk=NK,
    split_waits=True,
    xt_bufs=3,
    exp_bufs=5,
    sp_bufs=2,
    av_bufs=2,
    op_bufs=2,
    qsl_bufs=2,
    qtb_bufs=3,
    atn_bufs=8,
    rl_bufs=4,
    osb_bufs=9,
    warm_mms=13,
):
    """Build the per-core Bass module (SPMD; all cores run this program)."""
    nc = bass.Bass()

    q_h = nc.declare_dram_parameter("qT", [E, nq], BF16, isOutput=False)
    k_h = nc.declare_dram_parameter("kT", [E, nk], BF16, isOutput=False)
    v_h = nc.declare_dram_parameter("vT", [E, nk], BF16, isOutput=False)
    wq_h = nc.declare_dram_parameter("wq", [E, IL], BF16, isOutput=False)
    wk_h = nc.declare_dram_parameter("wk", [E, IL], BF16, isOutput=False)
    wv_h = nc.declare_dram_parameter("wv", [E, IL], BF16, isOutput=False)
    bq_h = nc.declare_dram_parameter("bq", [IL], F32, isOutput=False)
    bk_h = nc.declare_dram_parameter("bk", [IL], F32, isOutput=False)
    bv_h = nc.declare_dram_parameter("bv", [IL], F32, isOutput=False)
    wo_h = nc.declare_dram_parameter("wo", [IL, E], BF16, isOutput=False)
    out_h = nc.declare_dram_parameter("out", [nq, E], BF16, isOutput=True)

    EC = E // 128        # 8 E-chunks
    IB = IL // 128       # 4 INT-chunks (head pairs)
    KB = nk // 128       # 16 key blocks
    SC = nk // 512       # 4 key chunks (of 4 kb each)
    KPC = 4              # kb per chunk
    JQ = nq // 512
    SCALE = 1.0 / math.sqrt(HD)
    Exp = mybir.ActivationFunctionType.Exp

    with _SplitDrainTC(nc) as tc, ExitStack() as top:
        singles = top.enter_context(tc.tile_pool(name="singles", bufs=1))
        persist = top.enter_context(tc.tile_pool(name="persist", bufs=1))
        qtb_p = top.enter_context(tc.tile_pool(name="qtb", bufs=qtb_bufs))
        qsl_p = top.enter_context(tc.tile_pool(name="qsl", bufs=qsl_bufs))
        rld_p = top.enter_context(tc.tile_pool(name="rld", bufs=8, space="DRAM"))
        spsum = top.enter_context(
            tc.tile_pool(name="spsum", bufs=sp_bufs, space="PSUM"))
        avpsum = top.enter_context(
            tc.tile_pool(name="avpsum", bufs=av_bufs, space="PSUM"))
        opsum = top.enter_context(
            tc.tile_pool(name="opsum", bufs=op_bufs, space="PSUM"))

        # biases as [128 part, chunk] per-partition columns
        bq_sb = singles.tile([128, IB], F32)
        bk_sb = singles.tile([128, IB], F32)
        bv_sb = singles.tile([128, IB], F32)

        def load_biases():
            for b_sb, b_h in ((bq_sb, bq_h), (bk_sb, bk_h), (bv_sb, bv_h)):
                nc.scalar.dma_start(
                    out=b_sb, in_=b_h.ap().rearrange("(c p) -> p c", p=128)
                )
        negs = singles.tile([128, 1], F32)
        nc.vector.memset(negs, -EXP_SHIFT)
        ones64 = singles.tile([1, 64], BF16)
        nc.vector.memset(ones64, 1.0)
        warm = singles.tile([128, 512], BF16)
        nc.vector.memset(warm[:, 0:1], 0.0)
        wps = spsum.tile([128, 1024], F32, tag="s")
        for i in range(warm_mms):
            nc.tensor.matmul(wps[:, 0:512], warm[:, 0:128], warm,
                             start=True, stop=True)

        wqb = persist.tile([128, EC, IL], BF16)

        def load_wqb():
            nc.sync.dma_start(
                out=wqb, in_=wq_h.ap().rearrange("(c p) i -> p c i", p=128)
            )
        wo_sb = persist.tile([128, IB, E], BF16)

        def load_wo_chunk(ib):
            nc.sync.dma_start(
                out=wo_sb[:, ib, :],
                in_=wo_h.ap().rearrange("(c p) e -> p c e", p=128)[:, ib, :],
            )
        # k^T: [(h2, hd) part, head-pair, k] per sc chunk
        khTs = [persist.tile([128, IB, 512], BF16, name=f"khT_{i}")
                for i in range(SC)]
        # v: [k part, head, kb-in-chunk, 65] (col 64 = ones -> denominator)
        vhs = [persist.tile([128, HL, KPC, 65], BF16, name=f"vh_{i}")
               for i in range(SC)]
        for t in vhs:
            nc.vector.memset(t[:, :, :, 64:65], 1.0)

        qtb_tiles = {}

        def load_qTb(jq):
            qtb_tiles[jq] = qtb_p.tile([128, EC, 512], BF16, tag="qtb",
                                       name=f"qTb{jq}")
            eng = nc.sync
            eng.dma_start(
                out=qtb_tiles[jq],
                in_=q_h.ap().rearrange("(c p) n -> p c n", p=128)[
                    :, :, jq * 512 : jq * 512 + 512],
            )

        qsl_tiles = {}

        def qproj_chunk(jq, ib):
            """q-projection chunk ib of 512-q block jq -> qsl bf16 (+bias)."""
            if jq not in qsl_tiles:
                qsl_tiles[jq] = qsl_p.tile([128, IB, 512], BF16, tag="qsl",
                                           name=f"qsl{jq}")
            qsl = qsl_tiles[jq]
            ps = opsum.tile([128, 512], F32, tag="o")
            for ec in range(EC):
                nc.tensor.matmul(
                    ps,
                    wqb[:, ec, ib * 128 : ib * 128 + 128],
                    qtb_tiles[jq][:, ec, :],
                    start=(ec == 0),
                    stop=(ec == EC - 1),
                )
            nc.vector.tensor_scalar_add(
                out=qsl[:, ib, :], in0=ps, scalar1=bq_sb[:, ib : ib + 1])

        # ---------------- phase B: k/v projections ----------------
        with ExitStack() as phb:
            wpool = phb.enter_context(tc.tile_pool(name="weights", bufs=1))
            # wk in two ec-halves so the first kproj can start after half
            # the weight + half the first k chunk have landed
            wk_h1 = wpool.tile([128, EC // 2, IL], BF16)
            wk_h2 = wpool.tile([128, EC // 2, IL], BF16)
            wv_sb = wpool.tile([128, EC, IL], BF16)
            xt_p = phb.enter_context(tc.tile_pool(name="xT", bufs=xt_bufs))
            xth_p = phb.enter_context(tc.tile_pool(name="xTh", bufs=2))

            def load_w(w_sb, w_h):
                nc.sync.dma_start(
                    out=w_sb, in_=w_h.ap().rearrange("(c p) i -> p c i", p=128)
                )

            def load_w_half(w_sb, w_h, half):
                nc.sync.dma_start(
                    out=w_sb,
                    in_=w_h.ap().rearrange("(c p) i -> p c i", p=128)[
                        :, half * (EC // 2) : (half + 1) * (EC // 2), :],
                )

            def load_xts(src_h, sc):
                xt = xt_p.tile([128, EC, 512], BF16, tag="xt")
                nc.sync.dma_start(
                    out=xt,
                    in_=src_h.ap().rearrange("(c p) n -> p c n", p=128)[
                        :, :, sc * 512 : sc * 512 + 512],
                )
                return [xt[:, ec, :] for ec in range(EC)]

            def load_xts_halves(src_h, sc):
                parts = []
                for half in range(2):
                    xt = xth_p.tile([128, EC // 2, 512], BF16, tag="xth")
                    nc.sync.dma_start(
                        out=xt,
                        in_=src_h.ap().rearrange("(c p) n -> p c n", p=128)[
                            :, half * (EC // 2) : (half + 1) * (EC // 2),
                            sc * 512 : sc * 512 + 512],
                    )
                    parts.extend(xt[:, i, :] for i in range(EC // 2))
                return parts

            def kproj_chunk(sc, ib, xts):
                ps = opsum.tile([128, 512], F32, tag="o")
                for ec in range(EC):
                    wk_t = wk_h1 if ec < EC // 2 else wk_h2
                    nc.tensor.matmul(
                        ps,
                        wk_t[:, ec % (EC // 2), ib * 128 : ib * 128 + 128],
                        xts[ec],
                        start=(ec == 0),
                        stop=(ec == EC - 1),
                    )
                nc.vector.tensor_scalar_add(
                    out=khTs[sc][:, ib, :], in0=ps,
                    scalar1=bk_sb[:, ib : ib + 1])

            def vproj_chunk(sc, sb, xts):
                ps = opsum.tile([128, 512], F32, tag="o")
                for ec in range(EC):
                    nc.tensor.matmul(
                        ps,
                        xts[ec][:, sb * 128 : sb * 128 + 128],
                        wv_sb[:, ec, :],
                        start=(ec == 0),
                        stop=(ec == EC - 1),
                    )
                nc.vector.tensor_copy(
                    out=vhs[sc][:, :, sb, 0:64],
                    in_=ps.rearrange("p (h d) -> p h d", h=HL),
                )

            # DMA order: wk/kxt(0) in halves -> wv -> vxt(0) -> wqb/qTb(0)
            load_w_half(wk_h1, wk_h, 0)
            load_biases()
            for sc in range(SC):
                if sc == 0:
                    kxts = load_xts_halves(k_h, 0)
                    load_w_half(wk_h2, wk_h, 1)
                    load_w(wv_sb, wv_h)
                else:
                    kxts = load_xts(k_h, sc)
                vxts = load_xts(v_h, sc)
                if sc == 0:
                    load_wqb()
                    load_qTb(0)
                for ib in range(IB):
                    kproj_chunk(sc, ib, kxts)
                for sb in range(KPC):
                    vproj_chunk(sc, sb, vxts)
                if sc == 0:
                    for ib in range(IB):
                        qproj_chunk(0, ib)

        # ---------------- attention + output projection ----------------
        exp_p = top.enter_context(tc.tile_pool(name="exps", bufs=exp_bufs))
        atn_p = top.enter_context(tc.tile_pool(name="atn", bufs=atn_bufs))
        rl_p = top.enter_context(tc.tile_pool(name="rl", bufs=rl_bufs))
        out_p = top.enter_context(tc.tile_pool(name="outsb", bufs=osb_bufs))

        exp_tiles = {}
        atn_tiles = {}
        av_state = {}

        def qk_emit(jq, p_, kb):
            """Scores^T for both heads of pair p_ at key block kb, then exp."""
            sc, kk = kb // KPC, kb % KPC
            key = (jq, p_, sc)
            if key not in exp_tiles:
                exp_tiles[key] = exp_p.tile([128, KPC, 2, 512], BF16, tag="e",
                                            name=f"e{jq}_{p_}_{sc}")
            et = exp_tiles[key]
            ps = spsum.tile([128, 1024], F32, tag="s")
            qsl = qsl_tiles[jq]
            for h2 in (0, 1):
                hp = slice(h2 * 64, h2 * 64 + 64)
                nc.tensor.matmul(
                    ps[:, h2 * 512 : h2 * 512 + 512],
                    khTs[sc][hp, p_, kk * 128 : kk * 128 + 128],
                    qsl[hp, p_, :],
                    start=True,
                    stop=True,
                )
            nc.scalar.activation(out=_flat(et[:, kk, :, :], 1024), in_=ps,
                                 func=Exp, scale=SCALE, bias=negs[:, 0:1])

        av_psums = {}

        def av_stream(jq, p_, sc):
            """P@V matmuls for one key chunk, accumulating into held psums
            (used for the final pair so its AV rides inside the last slot)."""
            if (jq, p_) not in av_psums:
                av_psums[(jq, p_)] = [
                    avpsum.tile([65, 512], F32, tag="av",
                                name=f"avs{jq}_{p_}_{h2}")
                    for h2 in (0, 1)]
            for h2 in (0, 1):
                h = 2 * p_ + h2
                et = exp_tiles[(jq, p_, sc)]
                for kk in range(KPC):
                    nc.tensor.matmul(
                        av_psums[(jq, p_)][h2],
                        vhs[sc][:, h, kk, 0:65],
                        et[:, kk, h2, :],
                        start=(sc == 0 and kk == 0),
                        stop=(sc == SC - 1 and kk == KPC - 1),
                    )

        def av_start(jq, p_, bcast_pe=False):
            """P@V (ones-augmented) for both heads; evac + launch the 1/l
            DRAM-bounce broadcast. The multiply happens in av_finish two
            slots later so the DMA latency never stalls the DVE."""
            avsbs = []
            streamed = av_psums.pop((jq, p_), None)
            rl1 = rl_p.tile([1, 1024], BF16, tag="rl1")
            for h2 in (0, 1):
                h = 2 * p_ + h2
                if streamed is not None:
                    ps = streamed[h2]
                else:
                    ps = avpsum.tile([65, 512], F32, tag="av")
                    for kb in range(KB):
                        sc, kk = kb // KPC, kb % KPC
                        et = exp_tiles[(jq, p_, sc)]
                        nc.tensor.matmul(
                            ps,
                            vhs[sc][:, h, kk, 0:65],
                            et[:, kk, h2, :],
                            start=(kb == 0),
                            stop=(kb == KB - 1),
                        )
                # 1/l of the denom row first (both heads share one bounce
                # tile) so the broadcast launches before the bulk evac
                with nc.allow_low_precision(reason="1/l broadcast in bf16"):
                    nc.vector.reciprocal(
                        out=rl1[:, h2 * 512 : h2 * 512 + 512],
                        in_=ps[64:65, :])
                avsb = rl_p.tile([65, 512], BF16, tag="avsb")
                nc.vector.tensor_copy(out=avsb, in_=ps)
                avsbs.append(avsb)
            if bcast_pe:
                # tail path: broadcast 1/l over d with 1-row PE matmuls so
                # the final normalize never waits on a DRAM round-trip; the
                # av_finish multiply reads the factors straight from psum.
                # Output banks come from the (idle) opsum pool so nothing
                # waits on the exp stream to free a score buffer.
                rt = [opsum.tile([128, 512], F32, tag="o", name=f"rb{hf}")
                      for hf in range(2)]
                for hf in range(2):
                    nc.tensor.matmul(rt[hf][0:64, :],
                                     ones64, rl1[:, hf * 512 : hf * 512 + 512],
                                     start=True, stop=True)
                rlb = rt
            else:
                # bounce through DRAM; stride-0 partition AP broadcasts over d
                rld = rld_p.tile([1, 1024], BF16, tag="rld")
                nc.sync.dma_start(out=rld, in_=rl1)
                rlb = rl_p.tile([64, 1024], BF16, tag="rlb")
                nc.sync.dma_start(
                    out=rlb,
                    in_=bass.AP(
                        tensor=rld.tensor,
                        offset=rld.offset,
                        ap=[[0, 64]] + list(rld.ap)[1:],
                    ),
                )
            av_state[(jq, p_)] = (avsbs, rlb)
            for sc in range(SC):
                exp_tiles.pop((jq, p_, sc), None)

        def av_finish(jq, p_):
            atn = atn_p.tile([128, 512], BF16, tag="a", name=f"atn{jq}_{p_}")
            atn_tiles[(jq, p_)] = atn
            avsbs, rlb = av_state.pop((jq, p_))
            for h2, avsb in enumerate(avsbs):
                hrows = slice(h2 * 64, h2 * 64 + 64)
                fac = (rlb[h2][0:64, :] if isinstance(rlb, list)
                       else rlb[:, h2 * 512 : h2 * 512 + 512])
                nc.vector.tensor_mul(atn[hrows, :], avsb[0:64, :], fac)
            # + bv (deferred v-projection bias: P @ (vh + bv) = P@vh + l*bv,
            #   so atn just gains bv after normalize)
            nc.vector.tensor_scalar_add(
                out=atn, in0=atn, scalar1=bv_sb[:, p_ : p_ + 1])

        def oproj_chunk(jq, m2, split_store=False):
            m = jq * 4 + m2
            osb = out_p.tile([128, 1024], BF16, tag="osb")
            for half in range(2):
                ps = opsum.tile([128, 512], F32, tag="o")
                for ic in range(IB):
                    nc.tensor.matmul(
                        ps,
                        atn_tiles[(jq, ic)][:, m2 * 128 : m2 * 128 + 128],
                        wo_sb[:, ic, half * 512 : half * 512 + 512],
                        start=(ic == 0),
                        stop=(ic == IB - 1),
                    )
                nc.vector.tensor_copy(
                    out=osb[:, half * 512 : half * 512 + 512], in_=ps)
                if split_store:
                    nc.sync.dma_start(
                        out=out_h.ap()[m * 128 : m * 128 + 128,
                                       half * 512 : half * 512 + 512],
                        in_=osb[:, half * 512 : half * 512 + 512])
            if not split_store:
                nc.sync.dma_start(
                    out=out_h.ap()[m * 128 : m * 128 + 128, :], in_=osb)

        # pipelined jq loop over 16 linear slots t=(jq,p_): each slot
        # interleaves its 16 QK+exp with deferred work scheduled by slot
        # index: av_start one slot after its QK, av_finish two slots after
        # (hiding the 1/l DMA bounce), qproj for the next jq, and oproj
        # chunks after the owning jq's last av_finish.
        NSLOT = 4 * JQ
        fills = {}

        def at(t, fn):
            fills.setdefault(t, []).append(fn)

        load_qTb(1)
        for ib in range(IB):
            at(1 + ib, lambda ib=ib: load_wo_chunk(ib))
        at(2, lambda: load_qTb(2))
        at(6, lambda: load_qTb(3))

        for jq in range(JQ):
            for p_ in range(4):
                t = 4 * jq + p_
                fstart = (lambda jq=jq, p=p_, b=(t == NSLOT - 1):
                          av_start(jq, p, bcast_pe=b))
                fstart._av_last = 1
                at(t + 1, fstart)
                fin = t + 1 if t + 2 > NSLOT else t + 2
                at(fin, lambda jq=jq, p=p_: av_finish(jq, p))
            if jq + 1 < JQ:
                for ib in range(IB):
                    at(4 * jq + ib, lambda jq=jq + 1, ib=ib: qproj_chunk(jq, ib))
            for m2 in range(4):
                # last av_finish of jq lands at 4*jq+5; oproj from there
                # (m2=3 shares slot 4jq+7 so less lands in the tail)
                at(4 * jq + 5 + m2,
                   lambda jq=jq, m2=m2, ss=(jq == JQ - 1 and m2 == 3):
                   oproj_chunk(jq, m2, split_store=ss))

        qk_sched = {t: [(t // 4, t % 4, kb) for kb in range(KB)]
                    for t in range(NSLOT)}

        for t in range(NSLOT):
            jq, p_ = t // 4, t % 4
            todo = fills.pop(t, [])
            # av_start last within the slot: its P@V then has the whole
            # slot's exp production behind it before the next slot's QK
            todo.sort(key=lambda f: getattr(f, '_av_last', 0))
            items = qk_sched[t]
            if t == NSLOT - 1:
                # final slot: QK first so its exps drain before the tail
                for item in items:
                    qk_emit(*item)
                for fn in todo:
                    fn()
                continue
            fi = 0
            for ki, item in enumerate(items):
                qk_emit(*item)
                while (fi < len(todo)
                       and ki + 1 >= (fi + 1) * len(items) // (len(todo) + 1)):
                    todo[fi]()
                    fi += 1
            while fi < len(todo):
                todo[fi]()
                fi += 1
        for t in sorted(fills):
            for fn in fills.pop(t):
                fn()

    if split_waits:
        _split_waits(nc)
    return nc


_CACHED = {}


def _get_nc(nq=NQ, nk=NK):
    key = (nq, nk)
    if key not in _CACHED:
        _CACHED[key] = build_nc(nq, nk)
    return _CACHED[key]


def shard_inputs(q, k, v, wq, bq, wk, bk, wv, bv, wo):
    """8 per-core input maps: core c -> (batch c//2, head-group c%2)."""
    import ml_dtypes

    bf = ml_dtypes.bfloat16
    in_maps = []
    for c in range(N_CORES):
        b, g = c // 2, c % 2
        sl = slice(g * IL, (g + 1) * IL)
        in_maps.append(
            {
                "qT": np.ascontiguousarray(q[b].T).astype(bf),
                "kT": np.ascontiguousarray(k[b].T).astype(bf),
                "vT": np.ascontiguousarray(v[b].T).astype(bf),
                "wq": np.ascontiguousarray(wq[:, sl]).astype(bf),
                "wk": np.ascontiguousarray(wk[:, sl]).astype(bf),
                "wv": np.ascontiguousarray(wv[:, sl]).astype(bf),
                "bq": np.ascontiguousarray(bq[sl]),
                "bk": np.ascontiguousarray(bk[sl]),
                "bv": np.ascontiguousarray(bv[sl]),
                "wo": np.ascontiguousarray(wo[sl, :]).astype(bf),
            }
        )
    return in_maps


def kernel(q, k, v, wq, bq, wk, bk, wv, bv, wo, bo, _trace=False):
    from concourse.bass_utils import run_bass_kernel_spmd

    q, k, v = (np.asarray(x, np.float32) for x in (q, k, v))
    wq, bq, wk, bk, wv, bv, wo, bo = (
        np.asarray(x, np.float32) for x in (wq, bq, wk, bk, wv, bv, wo, bo)
    )
    nc = _get_nc()
    in_maps = shard_inputs(q, k, v, wq, bq, wk, bk, wv, bv, wo)
    try:
        res = run_bass_kernel_spmd(
            nc, in_maps, core_ids=list(range(N_CORES)), trace=_trace
        )
    except Exception:
        if not _trace:
            raise
        import traceback

        traceback.print_exc()
        print("trace run failed; retrying without trace", flush=True)
        res = run_bass_kernel_spmd(
            nc, in_maps, core_ids=list(range(N_CORES)), trace=False
        )
    parts = np.stack(
        [np.asarray(res.results[c]["out"], np.float32) for c in range(N_CORES)]
    )
    out = parts.reshape(B, 2, NQ, E).sum(axis=1) + bo[None, None, :]
    if _trace:
        kernel.last_results = res
    return out.astype(np.float32)
